# revision 57
# baseline (speedup 1.0000x reference)
"""Masked multi-head attention (CLS-token sparse attention) on 8 Trainium2
NeuronCores, data-parallel over batch (1 batch element per core).

Sparsity: the key mask is query-independent, so masked keys contribute
nothing.  The host gathers the ~513 unmasked keys per batch, pads to
NK=640, and the device only computes K/V projections, scores, exp and
attention*V over those 640 key slots (padding slots get a -1e9 bias so
exp()==0 and the fused denominator ignores them).

Per-core math (transposed layouts keep every matmul operand natural):
  x^T [c, n] for queries; xkv^T [c, j'] gathered keys.
  q^T = (wq*scale)^T-matmul;  k^T [o, j'];  v [j', o] natural.
  S^T[j', i] = k_h^T.T @ q_h^T   (K=64 on partitions; head pairs share the
                                  PE array via row groups 0:64 / 64:128)
  E = exp(S^T + bias[j'])        (ACT, per-partition bias)
  [O'^T ; denom] = [v_h | 1].T @ E   (M=65: head dim + denominator row)
  out_attn^T = O'^T * (1/denom)      (GPSIMD partition_broadcast + DVE mul)
  y^T = wproj^T.T @ out_attn^T + bproj

All matmul inputs bf16, PSUM fp32, softmax pipeline fp32.
"""

import numpy as np
import ml_dtypes

B, N, C, H, D = 8, 1024, 1024, 16, 64
P = 128
KC = C // P      # 8 contraction chunks
OC = C // P      # 8 output-channel chunks
NB = N // 512    # 2 query chunks of 512
NK = 640         # padded gathered-key count (mean ~513, +8 sigma safe)
NCORES = 8

_CACHE = {}


# Best HW-measured configuration (interleaved A/B, min-of-rounds):
# merged two-head score/exp tiles + staged AV evacuation + Q/K projection
# groups fed into the attention PE stream; PSUM: 2(pj) + 2x2(s2) + 1x2(av) = 8.
# norm2: denominator row 0 of the merged [65,1024] AV tile, single staging
# copy, immediate gpsimd broadcast of the raw denominator, deferred DVE
# divides (no reciprocal, no Pool->DVE head-block).
DEFAULT_OPTS = {"pj": 2, "sc": 2, "av": 1, "e": 10, "pipe": 1, "phase": "full",
                "emerge": 1, "feed": 1, "act_fn": "Exp", "noproj_post": 0,
                "norm2": 1, "vevac1": 1, "warm": 12, "dmaorder": 1,
                "stagepipe": 1, "st_bufs": 2, "bc_bufs": 2, "st_act": 1}

# test-time override hook: KOPTS='{"qk8":1}' python test.py
import os as _os
if _os.environ.get("KOPTS"):
    import json as _json
    DEFAULT_OPTS.update(_json.loads(_os.environ["KOPTS"]))


def _build_nc(repeat=1, nk=NK, opts=None):
    import concourse.bass as bass
    import concourse.tile as tile
    from concourse import bacc, mybir
    from contextlib import nullcontext, ExitStack
    opts = {**DEFAULT_OPTS, **(opts or {})}
    if opts.get("qk8"):
        assert opts.get("stagepipe") and nk // P == 5, \
            "qk8 requires the stage-pipelined attention (nk=640)"

    bf16 = mybir.dt.bfloat16
    f32 = mybir.dt.float32

    nc = bacc.Bacc("TRN2", target_bir_lowering=False, debug=False)

    xt_d = nc.dram_tensor("xt", [C, N], bf16, kind="ExternalInput").ap()
    xkv_d = nc.dram_tensor("xkv", [C, nk], bf16, kind="ExternalInput").ap()
    wqt_d = nc.dram_tensor("wqt", [C, C], bf16, kind="ExternalInput").ap()
    wkt_d = nc.dram_tensor("wkt", [C, C], bf16, kind="ExternalInput").ap()
    wvt_d = nc.dram_tensor("wvt", [C, C], bf16, kind="ExternalInput").ap()
    wpt_d = nc.dram_tensor("wpt", [C, C], bf16, kind="ExternalInput").ap()
    mb_d = nc.dram_tensor("mb", [nk], f32, kind="ExternalInput").ap()
    bb_d = nc.dram_tensor("bb", [C], f32, kind="ExternalInput").ap()
    yt_d = nc.dram_tensor("yt", [C, N], f32, kind="ExternalOutput").ap()

    with tile.TileContext(nc) as tc:
        with ExitStack() as ctx:
            pools = _make_pools(tc, ctx, opts)
            tiles = _load_inputs(nc, tc, mybir, pools, nk, xt_d, xkv_d, wqt_d,
                                 wkt_d, wvt_d, wpt_d, mb_d, bb_d, opts)
            if repeat > 1 and not opts.get("unroll"):
                from concourse.engine_type import EngineType
                hints = (EngineType.PE, EngineType.Activation, EngineType.DVE,
                         EngineType.Pool, EngineType.SP)
                loop = tc.For_i(0, repeat, 1, hint_engines=hints)
            else:
                loop = nullcontext()
            phase = opts["phase"]
            if phase == "full":
                with loop:
                    for _ in range(repeat if opts.get("unroll") else 1):
                        _compute(nc, tc, mybir, pools, nk, tiles, yt_d, opts)
            else:
                # phase-isolation timing builds: put one sub-phase in the
                # repeat loop, run the others once around it
                args = (nc, tc, mybir, pools, nk, tiles, yt_d, opts)
                unroll_n = repeat if opts.get("unroll") else 1
                if phase == "qkv":
                    with loop:
                        for _ in range(unroll_n):
                            _compute_qkv(*args)
                    _compute_attn(*args)
                    _compute_proj(*args)
                elif phase == "attn":
                    _compute_qkv(*args)
                    with loop:
                        for _ in range(unroll_n):
                            _compute_attn(*args)
                    _compute_proj(*args)
                elif phase == "proj":
                    _compute_qkv(*args)
                    _compute_attn(*args)
                    with loop:
                        for _ in range(unroll_n):
                            _compute_proj(*args)
                else:
                    raise ValueError(phase)
    nc.compile()
    return nc


def _make_pools(tc, ctx, opts=None):
    opts = {**DEFAULT_OPTS, **(opts or {})}
    return {
        "const": ctx.enter_context(tc.tile_pool(name="const", bufs=1)),
        "e": ctx.enter_context(tc.tile_pool(name="e", bufs=opts["e"])),
        "recip": ctx.enter_context(
            tc.tile_pool(name="recip", bufs=opts.get("r_bufs", 4))),
        "bcast": ctx.enter_context(
            tc.tile_pool(name="bcast", bufs=opts.get("bc_bufs", 3))),
        "yt": ctx.enter_context(
            tc.tile_pool(name="yt", bufs=opts.get("yt_bufs", 3))),
        "stage": ctx.enter_context(
            tc.tile_pool(name="stage", bufs=opts.get("st_bufs", 3))),
        "q8s": ctx.enter_context(tc.tile_pool(name="q8s", bufs=4)),
        "pj_ps": ctx.enter_context(
            tc.tile_pool(name="pj_ps", bufs=opts["pj"], space="PSUM")),
        "sc_ps": ctx.enter_context(
            tc.tile_pool(name="sc_ps", bufs=opts["sc"], space="PSUM")),
        "av_ps": ctx.enter_context(
            tc.tile_pool(name="av_ps", bufs=opts["av"], space="PSUM")),
    }


def _load_inputs(nc, tc, mybir, pools, nk, xt_d, xkv_d, wqt_d, wkt_d, wvt_d,
                 wpt_d, mb_d, bb_d, opts=None):
    opts = {**DEFAULT_OPTS, **(opts or {})}
    bf16 = mybir.dt.bfloat16
    f32 = mybir.dt.float32
    const = pools["const"]
    jchunks = nk // P

    xt = const.tile([P, KC, N], bf16)       # x^T   [p, kc, n]
    xkv = const.tile([P, KC, nk], bf16)     # gathered keys x^T [p, kc, j']
    wqt = const.tile([P, KC, C], bf16)      # wq^T  [p, kc, o]  (pre-scaled)
    wkt = const.tile([P, KC, C], bf16)
    wvt = const.tile([P, KC, C], bf16)
    wpt = const.tile([P, KC, C], bf16)
    mb = const.tile([P, jchunks], f32)      # bias per key slot (0 / -1e9 pad)
    bb = const.tile([P, OC], f32)           # proj bias per out channel o
    if opts.get("qk8"):
        # fp8e4 DoubleRow score layout: partition = h*32+ki, free (g, ko, n)
        # with the head dim d mapped as d = ko*32 + ki (same bijection for q
        # and k, so the contraction sum is unchanged).
        f8 = mybir.dt.float8e4
        q8 = const.tile([64, OC, 2, N], f8)
        k8 = const.tile([64, OC, 2, nk], f8)
        qt = kt = None
    else:
        q8 = k8 = None
        qt = const.tile([P, OC, N], bf16)   # q^T [p(o), oc, n]
        kt = const.tile([P, OC, nk], bf16)  # k^T [p(o), oc, j']
    _vs = 128 if opts.get("avpad") else 65
    vh = const.tile([P, jchunks, _vs * H], bf16)  # [p(j'), jc, vs*h+dd]
    oa = const.tile([P, KC, N], bf16)       # out_attn^T [p(c), cc, n]

    warm_n = opts.get("warm", 0)
    if warm_n:
        # PE warm-up: garbage matmuls from a zeroed tile keep the tensor
        # engine busy during the input-DMA window so HAM reaches 8/8 before
        # real compute starts (results discarded; pj pool recycles banks).
        wsrc = const.tile([P, 512], bf16)
        nc.vector.memset(wsrc, 0.0)
        f32_ = f32
        for w in range(warm_n):
            wps = pools["pj_ps"].tile([P, 512], f32_, name=f"warm_{w}",
                                      tag="pj")
            nc.tensor.matmul(wps, wsrc[:, 0:P], wsrc, start=True, stop=True)

    if opts.get("dmaorder"):
        # V-proj inputs first so compute starts ~10us into the load, then
        # Q-proj inputs, then the rest in first-use order.
        nc.gpsimd.dma_start(out=xkv, in_=xkv_d.rearrange("(k p) n -> p k n", p=P))
        nc.gpsimd.dma_start(out=wvt, in_=wvt_d.rearrange("(k p) o -> p k o", p=P))
        nc.gpsimd.dma_start(out=wqt, in_=wqt_d.rearrange("(k p) o -> p k o", p=P))
        nc.gpsimd.dma_start(out=xt, in_=xt_d.rearrange("(k p) n -> p k n", p=P))
        nc.gpsimd.dma_start(out=wkt, in_=wkt_d.rearrange("(k p) o -> p k o", p=P))
        nc.gpsimd.dma_start(out=mb, in_=mb_d.rearrange("(k p) -> p k", p=P))
        nc.gpsimd.dma_start(out=wpt, in_=wpt_d.rearrange("(k p) o -> p k o", p=P))
        nc.gpsimd.dma_start(out=bb, in_=bb_d.rearrange("(k p) -> p k", p=P))
    else:
        nc.gpsimd.dma_start(out=xt, in_=xt_d.rearrange("(k p) n -> p k n", p=P))
        nc.gpsimd.dma_start(out=xkv, in_=xkv_d.rearrange("(k p) n -> p k n", p=P))
        nc.gpsimd.dma_start(out=wvt, in_=wvt_d.rearrange("(k p) o -> p k o", p=P))
        nc.gpsimd.dma_start(out=wkt, in_=wkt_d.rearrange("(k p) o -> p k o", p=P))
        nc.gpsimd.dma_start(out=wqt, in_=wqt_d.rearrange("(k p) o -> p k o", p=P))
        nc.gpsimd.dma_start(out=wpt, in_=wpt_d.rearrange("(k p) o -> p k o", p=P))
        nc.gpsimd.dma_start(out=mb, in_=mb_d.rearrange("(k p) -> p k", p=P))
        nc.gpsimd.dma_start(out=bb, in_=bb_d.rearrange("(k p) -> p k", p=P))

    # ones columns of vh (denominator trick), one strided memset per jc.
    # The denominator stays at slot 64: partition 64 of the AV output is
    # 32-aligned, which engine partition-range rules require.  With avpad,
    # slots 65..127 are zeroed so the stationary is a full 128 columns
    # (FWL-eligible ldweights, full-bank output).
    vs = 128 if opts.get("avpad") else 65
    vh_r = vh.rearrange("p j (h e) -> p j h e", e=vs)
    for jc in range(jchunks):
        nc.vector.memset(vh_r[:, jc, :, 64], 1.0)
        if vs == 128:
            nc.vector.memset(vh_r[:, jc, :, 65:128], 0.0)

    if opts.get("memset_oa"):     # timing-only variants that skip oa writes
        nc.vector.memset(oa, 0.0)

    tiles = {"xt": xt, "xkv": xkv, "wqt": wqt, "wkt": wkt, "wvt": wvt,
             "wpt": wpt, "mb": mb, "bb": bb, "qt": qt, "kt": kt, "vh": vh,
             "oa": oa, "q8": q8, "k8": k8}
    if opts.get("act2"):          # timing probe sink for the dummy exps
        edump = const.tile([P, 1024], bf16)
        tiles["edump"] = edump
    return tiles


def _compute_qkv(nc, tc, mybir, pools, nk, t, yt_d, opts=None):
    """Phase-isolation build: V projection + all Q/K projection chunks."""
    opts = {**DEFAULT_OPTS, **(opts or {})}
    f32 = mybir.dt.float32
    xt, xkv, wqt, wkt, wvt = t["xt"], t["xkv"], t["wqt"], t["wkt"], t["wvt"]
    qt, kt, vh = t["qt"], t["kt"], t["vh"]
    pj_ps = pools["pj_ps"]
    JCH = nk // P
    for nb2 in range(2):
        for mc in range(JCH):
            ps = pj_ps.tile([P, 512], f32, name=f"v_{nb2}_{mc}", tag="pj")
            for kc in range(KC):
                nc.tensor.matmul(
                    ps, xkv[:, kc, mc * P:(mc + 1) * P],
                    wvt[:, kc, nb2 * 512:(nb2 + 1) * 512],
                    start=(kc == 0), stop=(kc == KC - 1))
            _v_evac(nc, vh, ps, nb2, mc, opts)
    for g in range(OC):
        for nb2 in range(NB):
            ps = pj_ps.tile([P, 512], f32, name=f"q_{g}_{nb2}", tag="pj")
            for kc in range(KC):
                nc.tensor.matmul(
                    ps, wqt[:, kc, g * P:(g + 1) * P],
                    xt[:, kc, nb2 * 512:(nb2 + 1) * 512],
                    start=(kc == 0), stop=(kc == KC - 1))
            if opts.get("qk8"):
                _qk_evac8(nc, mybir, pools, ps, t["q8"], g, nb2 * 512, 512,
                          name=f"q8s_{g}_{nb2}")
            else:
                nc.vector.tensor_copy(
                    qt[:, g, nb2 * 512:(nb2 + 1) * 512], ps)
        for j0, jw in ((0, 512), (512, nk - 512)):
            ps = pj_ps.tile([P, jw], f32, name=f"k_{g}_{j0}", tag="pj")
            for kc in range(KC):
                nc.tensor.matmul(
                    ps, wkt[:, kc, g * P:(g + 1) * P],
                    xkv[:, kc, j0:j0 + jw],
                    start=(kc == 0), stop=(kc == KC - 1))
            if opts.get("qk8"):
                _qk_evac8(nc, mybir, pools, ps, t["k8"], g, j0, jw,
                          name=f"k8s_{g}_{j0}")
            else:
                nc.vector.tensor_copy(kt[:, g, j0:j0 + jw], ps)


def _compute_attn(nc, tc, mybir, pools, nk, t, yt_d, opts=None):
    """Phase-isolation build: all head-pair attention."""
    opts = {**DEFAULT_OPTS, **(opts or {})}
    if opts.get("stagepipe") and nk // P == 5 \
            and opts.get("attn_var", "full") == "full":
        _attn_pipelined(nc, tc, mybir, pools, nk, t, opts, [])
        return
    for g in range(OC):
        _attn_pair(nc, tc, mybir, pools, nk, t, g, opts)


def _compute_proj(nc, tc, mybir, pools, nk, t, yt_d, opts=None):
    opts = {**DEFAULT_OPTS, **(opts or {})}
    f32 = mybir.dt.float32
    wpt, oa, bb = t["wpt"], t["oa"], t["bb"]
    pj_ps, y_pool = pools["pj_ps"], pools["yt"]
    for oc in range(OC):
        for nb2 in range(NB):
            ps = pj_ps.tile([P, 512], f32, name=f"y_{oc}_{nb2}", tag="pj")
            for kc in range(KC):
                nc.tensor.matmul(
                    ps, wpt[:, kc, oc * P:(oc + 1) * P],
                    oa[:, kc, nb2 * 512:(nb2 + 1) * 512],
                    start=(kc == 0), stop=(kc == KC - 1))
            if opts.get("noproj_post"):
                yt = y_pool.tile([P, 8], f32, name=f"yt_{oc}_{nb2}", tag="yt")
                nc.vector.tensor_copy(yt, ps[:, 0:8])
                nc.gpsimd.dma_start(
                    out=yt_d[oc * P:(oc + 1) * P, nb2 * 8:(nb2 + 1) * 8],
                    in_=yt)
                continue
            yt = y_pool.tile([P, 512], f32, name=f"yt_{oc}_{nb2}", tag="yt")
            nc.vector.tensor_scalar_add(yt, ps, bb[:, oc:oc + 1])
            nc.gpsimd.dma_start(
                out=yt_d[oc * P:(oc + 1) * P, nb2 * 512:(nb2 + 1) * 512],
                in_=yt)


def _v_evac(nc, vh, ps, nb2, mc, opts):
    """PSUM -> vh evacuation for one V-projection tile (8 heads x 64 dims).
    vevac1: single strided copy; else 8 per-head copies."""
    vs = 128 if opts.get("avpad") else 65
    if opts.get("vevac1"):
        vh_r = vh.rearrange("p j (h e) -> p j h e", e=vs)
        ps_r = ps.rearrange("p (h e) -> p h e", e=64)
        nc.vector.tensor_copy(
            vh_r[:, mc, nb2 * 8:(nb2 + 1) * 8, 0:64], ps_r)
    else:
        for hh in range(8):
            h = nb2 * 8 + hh
            nc.vector.tensor_copy(
                vh[:, mc, vs * h:vs * h + 64],
                ps[:, hh * 64:(hh + 1) * 64])


def _qk_evac8(nc, mybir, pools, ps, dst8, g, col0, width, name):
    """fp8 evacuation of a Q/K projection psum tile into the DoubleRow
    layout dst8[h*32+ki, g, ko, col0:].  The host permutes wq/wk columns to
    (ko, h, ki) order, so psum partitions 0:64 are the ko=0 plane and
    64:128 the ko=1 plane — two plain (base-shifted) DVE copies suffice."""
    nc.vector.tensor_copy(dst8[:, g, 0, col0:col0 + width], ps[0:64, :])
    nc.vector.tensor_copy(dst8[:, g, 1, col0:col0 + width], ps[64:128, :])


def _attn_pair(nc, tc, mybir, pools, nk, t, g, opts):
    if opts.get("emerge"):
        return _attn_pair_merged(nc, tc, mybir, pools, nk, t, g, opts)
    bf16 = mybir.dt.bfloat16
    f32 = mybir.dt.float32
    _vhs = 128 if opts.get("avpad") else 65
    Exp = mybir.ActivationFunctionType.Exp
    mb, qt, kt, vh, oa = t["mb"], t["qt"], t["kt"], t["vh"], t["oa"]
    e_pool, r_pool, bc_pool = pools["e"], pools["recip"], pools["bcast"]
    sc_ps, av_ps = pools["sc_ps"], pools["av_ps"]
    JCH = nk // P
    ha, hb = 2 * g, 2 * g + 1
    for ic in range(NB):
        i0 = ic * 512
        avs = {}
        for h, p0 in ((ha, 0), (hb, 64)):
            avs[h] = av_ps.tile([65, 512], f32, name=f"av_{h}_{ic}", tag="av")
        edict = {}

        def scores_chunk(jc):
            for h, p0 in ((ha, 0), (hb, 64)):
                s_ps = sc_ps.tile([P, 512], f32, name=f"s_{h}_{ic}_{jc}",
                                  tag="s")
                nc.tensor.matmul(
                    s_ps,
                    kt[p0:p0 + 64, g, jc * P:(jc + 1) * P],
                    qt[p0:p0 + 64, g, i0:i0 + 512],
                    start=True, stop=True)
                e = e_pool.tile([P, 512], bf16, name=f"e_{h}_{ic}_{jc}",
                                tag="e")
                nc.scalar.activation(e, s_ps, Exp, bias=mb[:, jc:jc + 1])
                edict[(h, jc)] = e

        if opts["pipe"]:
            scores_chunk(0)
            for jc in range(JCH):
                if jc + 1 < JCH:
                    scores_chunk(jc + 1)
                for h in (ha, hb):
                    nc.tensor.matmul(
                        avs[h], vh[:, jc, _vhs * h:_vhs * h + 65],
                        edict.pop((h, jc)),
                        start=(jc == 0), stop=(jc == JCH - 1))
        else:
            for jc in range(JCH):
                scores_chunk(jc)
                for h in (ha, hb):
                    nc.tensor.matmul(
                        avs[h], vh[:, jc, _vhs * h:_vhs * h + 65],
                        edict.pop((h, jc)),
                        start=(jc == 0), stop=(jc == JCH - 1))
        for h, p0 in ((ha, 0), (hb, 64)):
            recip = r_pool.tile([1, 512], f32, name=f"r_{h}_{ic}", tag="r")
            nc.vector.reciprocal(recip, avs[h][64:65, :])
            bc = bc_pool.tile([64, 512], f32, name=f"bc_{h}_{ic}", tag="bc")
            nc.gpsimd.partition_broadcast(bc, recip)
            nc.vector.tensor_mul(
                oa[p0:p0 + 64, g, i0:i0 + 512], avs[h][0:64, :], bc)


def _attn_pair_merged(nc, tc, mybir, pools, nk, t, g, opts, feeder=None,
                      pending=None):
    """Attention for head pair (2g, 2g+1) with both heads' scores in one
    2-bank PSUM tile -> single [128, 1024] exp per key chunk (halves ACT
    per-op overhead), and AV results staged out of PSUM through SBUF with a
    single copy so the banks recycle without waiting for the normalize
    chain (reciprocal -> partition_broadcast -> multiply).

    feeder: optional iterator of zero-arg callables, each emitting one
    independent PE work unit (a projection PSUM group).  They are injected
    between score chunks so the in-order PE stream has useful work while
    waiting on ACT exp results.

    pending: optional single-slot list carrying the deferred normalize tail
    (reciprocal -> broadcast -> multiply) of the previous (g, ic) unit.  The
    tail is emitted in the middle of the NEXT unit's AV stream so the
    in-order DVE/Pool queues never delay the PSUM-freeing st copies of the
    unit in flight."""
    bf16 = mybir.dt.bfloat16
    f32 = mybir.dt.float32
    Exp = getattr(mybir.ActivationFunctionType, opts.get("act_fn", "Exp"))
    mb, qt, kt, vh, oa = t["mb"], t["qt"], t["kt"], t["vh"], t["oa"]
    e_pool, r_pool, bc_pool = pools["e"], pools["recip"], pools["bcast"]
    st_pool = pools["stage"]
    sc_ps, av_ps = pools["sc_ps"], pools["av_ps"]
    JCH = nk // P
    ha, hb = 2 * g, 2 * g + 1
    var = opts.get("attn_var", "full")
    _vhs = 128 if opts.get("avpad") else 65
    act_n = opts.get("act_elems", 1024)
    norm2 = opts.get("norm2")
    for ic in range(NB):
        i0 = ic * 512
        avs = {}
        av2 = None
        if var != "scoresonly":
            if norm2:
                av2 = av_ps.tile([65, 1024], f32, name=f"av2_{ic}", tag="av")
                avs = {ha: av2[:, 0:512], hb: av2[:, 512:1024]}
            else:
                for h, p0 in ((ha, 0), (hb, 64)):
                    avs[h] = av_ps.tile([65, 512], f32, name=f"av_{h}_{ic}",
                                        tag="av")
        edict = {}

        def scores_chunk(jc):
            s2 = sc_ps.tile([P, 1024], f32, name=f"s2_{ic}_{jc}", tag="s2")
            for (h, p0), c0 in (((ha, 0), 0), ((hb, 64), 512)):
                nc.tensor.matmul(
                    s2[:, c0:c0 + 512],
                    kt[p0:p0 + 64, g, jc * P:(jc + 1) * P],
                    qt[p0:p0 + 64, g, i0:i0 + 512],
                    start=True, stop=True)
            e2 = e_pool.tile([P, 1024], bf16, name=f"e2_{ic}_{jc}", tag="e")
            bias = 0.0 if opts.get("bias0") else mb[:, jc:jc + 1]
            nc.scalar.activation(e2[:, :act_n], s2[:, :act_n], Exp, bias=bias)
            edict[jc] = e2

        def av_chunk(jc):
            if var == "avonly":
                e2 = None
            else:
                e2 = edict.pop(jc)
            for h, c0 in ((ha, 0), (hb, 512)):
                mov = (qt[:, g, i0:i0 + 512] if e2 is None
                       else e2[:, c0:c0 + 512])
                nc.tensor.matmul(
                    avs[h], vh[:, jc, _vhs * h:_vhs * h + 65],
                    mov,
                    start=(jc == 0), stop=(jc == JCH - 1))

        def feed():
            if feeder is not None:
                unit = next(feeder, None)
                if unit is not None:
                    unit()

        def flush_pending():
            if pending and pending[0] is not None:
                fin = pending[0]
                pending[0] = None
                fin()

        if var == "avonly":
            for jc in range(JCH):
                av_chunk(jc)
                if jc == 1 and not norm2:
                    flush_pending()
        elif var == "scoresonly":
            for jc in range(JCH):
                scores_chunk(jc)
                e2 = edict.pop(jc)
                sink = st_pool.tile([P, 8], f32, name=f"sk_{ic}_{jc}",
                                    tag="st")
                nc.vector.tensor_copy(sink, e2[:, 0:8])
        elif opts["pipe"]:
            scores_chunk(0)
            for jc in range(JCH):
                if jc + 1 < JCH:
                    scores_chunk(jc + 1)
                if jc in (0, 2):
                    feed()
                av_chunk(jc)
                if jc == 1 and not norm2:
                    flush_pending()
        else:
            for jc in range(JCH):
                scores_chunk(jc)
                av_chunk(jc)
        if var == "scoresonly":
            continue
        norm = opts.get("norm_var", "full")

        if norm2:
            # Deferred divides of the previous unit go first: their Pool
            # broadcast finished long ago, so they retire without blocking
            # the staging copy that frees this unit's AV banks.
            flush_pending()
            st = st_pool.tile([65, 1024], f32, name=f"st2_{ic}", tag="st")
            nc.vector.tensor_copy(st, av2)       # frees both AV banks
            bc = bc_pool.tile([64, 1024], f32, name=f"bc2_{ic}", tag="bc")
            if norm != "none":
                recip = r_pool.tile([1, 1024], f32, name=f"r2_{ic}", tag="r")
                nc.vector.reciprocal(recip, st[64:65, :])
                nc.gpsimd.partition_broadcast(bc, recip)

            def finish(g=g, i0=i0, st=st, bc=bc):
                if norm == "none":
                    return
                for p0, c0 in ((0, 0), (64, 512)):
                    nc.vector.tensor_mul(
                        oa[p0:p0 + 64, g, i0:i0 + 512],
                        st[0:64, c0:c0 + 512], bc[0:64, c0:c0 + 512])

            if pending is None:
                finish()
            else:
                pending[0] = finish
            continue

        sts = {}
        for h, p0 in ((ha, 0), (hb, 64)):
            st = st_pool.tile([65, 512], f32, name=f"st_{h}_{ic}", tag="st")
            nc.vector.tensor_copy(st, avs[h])    # frees the PSUM bank fast
            sts[h] = st

        def finish(g=g, ic=ic, i0=i0, sts=sts):
            if norm == "none":
                return
            for h, p0 in ((ha, 0), (hb, 64)):
                st = sts[h]
                recip = r_pool.tile([1, 512], f32, name=f"r_{h}_{ic}",
                                    tag="r")
                nc.vector.reciprocal(recip, st[64:65, :])
                if norm == "norecip":
                    continue
                bc = bc_pool.tile([64, 512], f32, name=f"bc_{h}_{ic}",
                                  tag="bc")
                if norm == "dvebcast":
                    nc.vector.tensor_copy(bc, st[0:64, :])
                else:
                    nc.gpsimd.partition_broadcast(bc, recip)
                if norm == "nomul":
                    continue
                nc.vector.tensor_mul(
                    oa[p0:p0 + 64, g, i0:i0 + 512], st[0:64, :], bc)

        if pending is None:
            finish()
        else:
            flush_pending()          # in case previous tail still queued
            pending[0] = finish


def _attn_pipelined(nc, tc, mybir, pools, nk, t, opts, feeds):
    """One-stage software pipeline over the 16 (g, ic) units: unit u's
    scores+exp are emitted interleaved with unit u-1's AV+normalize, so no
    PE instruction ever waits on an ACT result issued the same step.  PSUM:
    pj(2) + 2x s2(4) + 1x merged av(2) = 8 banks.  e pool must hold two
    units' exp tiles (bufs >= 10)."""
    bf16 = mybir.dt.bfloat16
    f32 = mybir.dt.float32
    Exp = getattr(mybir.ActivationFunctionType, opts.get("act_fn", "Exp"))
    mb, qt, kt, vh, oa = t["mb"], t["qt"], t["kt"], t["vh"], t["oa"]
    e_pool, r_pool, bc_pool = pools["e"], pools["recip"], pools["bcast"]
    st_pool = pools["stage"]
    sc_ps, av_ps = pools["sc_ps"], pools["av_ps"]
    JCH = nk // P
    units = [(g, ic) for g in range(OC) for ic in range(NB)]
    U = len(units)
    e_store = {u: [] for u in range(U)}
    av_store = {}
    pending = [None]
    feeds = list(feeds)
    fi = [0]

    def feed():
        if fi[0] < len(feeds):
            feeds[fi[0]]()
            fi[0] += 1

    q8, k8 = t.get("q8"), t.get("k8")
    DoubleRow = mybir.MatmulPerfMode.DoubleRow

    def emit_sc(u, jc):
        g, ic = units[u]
        i0 = ic * 512
        s2 = sc_ps.tile([P, 1024], f32, name=f"s2_{u}_{jc}", tag="s2")
        if opts.get("qk8"):
            for hh, c0 in ((0, 0), (1, 512)):
                p0 = hh * 32
                nc.tensor.matmul(
                    s2[:, c0:c0 + 512],
                    k8[p0:p0 + 32, g, :, jc * P:(jc + 1) * P],
                    q8[p0:p0 + 32, g, :, i0:i0 + 512],
                    start=True, stop=True,
                    perf_mode=DoubleRow, tile_position=(p0, 0))
        else:
            for (h, p0), c0 in (((2 * g, 0), 0), ((2 * g + 1, 64), 512)):
                nc.tensor.matmul(
                    s2[:, c0:c0 + 512],
                    kt[p0:p0 + 64, g, jc * P:(jc + 1) * P],
                    qt[p0:p0 + 64, g, i0:i0 + 512],
                    start=True, stop=True)
        if opts.get("attn_var") == "noexp":   # timing probe: no ACT at all
            e_store[u].append(None)
            return
        e2 = e_pool.tile([P, 1024], bf16, name=f"e_{u}_{jc}", tag="e")
        nc.scalar.activation(e2, s2, Exp, bias=mb[:, jc:jc + 1])
        if opts.get("act2"):        # timing probe: double the ACT load
            nc.scalar.activation(t["edump"], e2, Exp)
        e_store[u].append(e2)

    vs = 128 if opts.get("avpad") else 65
    avp = vs if opts.get("avpad") else 65    # AV output partition count

    def emit_av(u, jc, only_hh=None):
        g, ic = units[u]
        if u not in av_store:
            av_store[u] = av_ps.tile([avp, 1024], f32, name=f"av2_{u}",
                                     tag="av")
        av2 = av_store[u]
        e2 = e_store[u][jc]
        i0 = ic * 512
        for hh, (h, c0) in enumerate(((2 * g, 0), (2 * g + 1, 512))):
            if only_hh is not None and hh != only_hh:
                continue
            mov = (qt[:, g, i0:i0 + 512] if e2 is None
                   else e2[:, c0:c0 + 512])
            nc.tensor.matmul(
                av2[:, c0:c0 + 512], vh[:, jc, vs * h:vs * h + avp],
                mov,
                start=(jc == 0), stop=(jc == JCH - 1))

    def flush_pending():
        if pending[0] is not None:
            fin = pending[0]
            pending[0] = None
            fin()

    def emit_norm(u):
        g, ic = units[u]
        i0 = ic * 512
        av2 = av_store.pop(u)
        e_store[u] = None
        if opts.get("nost"):
            # Normalize straight out of PSUM: no staging copy at all.  The
            # AV banks stay held until the muls retire, but the whole
            # st->ACT-queue coupling disappears.
            recip = r_pool.tile([1, 1024], f32, name=f"r2_{u}", tag="r")
            nc.vector.reciprocal(recip, av2[64:65, :])
            bc = bc_pool.tile([64, 1024], f32, name=f"bc2_{u}", tag="bc")
            nc.gpsimd.partition_broadcast(bc, recip)
            for p0, c0 in ((0, 0), (64, 512)):
                nc.vector.tensor_mul(
                    oa[p0:p0 + 64, g, i0:i0 + 512],
                    av2[0:64, c0:c0 + 512], bc[0:64, c0:c0 + 512])
            return
        st = st_pool.tile([65, 1024], f32, name=f"st2_{u}", tag="st")
        if opts.get("st_act"):
            # Stage on the Scalar engine: it has slack, reads PSUM fast, and
            # frees the AV banks without queuing behind DVE's normalize work.
            nc.scalar.copy(st, av2[0:65, :])
        else:
            nc.vector.tensor_copy(st, av2[0:65, :])
        recip = r_pool.tile([1, 1024], f32, name=f"r2_{u}", tag="r")
        nc.vector.reciprocal(recip, st[64:65, :])
        bc = bc_pool.tile([64, 1024], f32, name=f"bc2_{u}", tag="bc")
        nc.gpsimd.partition_broadcast(bc, recip)

        def finish(g=g, i0=i0, st=st, bc=bc):
            for p0, c0 in ((0, 0), (64, 512)):
                nc.vector.tensor_mul(
                    oa[p0:p0 + 64, g, i0:i0 + 512],
                    st[0:64, c0:c0 + 512], bc[0:64, c0:c0 + 512])

        pending[0] = finish

    weave = opts.get("weave", "fine")
    lag = opts.get("lag", 1)
    for step in range(U + lag):
        su = step if step < U else None      # unit doing scores+exp
        au = step - lag if step >= lag else None  # unit doing AV+norm
        if weave == "batch4":
            # batch3 + the norm (ACT staging copy) emitted right after the
            # AV block: st lands between exp1 and exp2 in the ACT queue, so
            # the next step's AV block (av bufs=1) never waits on it.
            if su is not None:
                emit_sc(su, 0)
                emit_sc(su, 1)
            if au is not None:
                if opts.get("avord") == "head":
                    for hh in (0, 1):
                        for jc in range(JCH):
                            emit_av(au, jc, only_hh=hh)
                else:
                    for jc in range(JCH):
                        emit_av(au, jc)
                flush_pending()
                emit_norm(au)
            feed()
            feed()
            if su is not None:
                emit_sc(su, 2)
                emit_sc(su, 3)
                emit_sc(su, 4)
            continue
        if weave == "batch3":
            # Cluster by PE tile config: row-tiled SCs in two blocks, all
            # full-array work (AV + projection feeds) contiguous between
            # them -> 2 config transitions per step instead of ~10, so
            # ldweights keeps overlapping via the background weight buffer.
            if su is not None:
                emit_sc(su, 0)
                emit_sc(su, 1)
            if au is not None:
                if opts.get("avord") == "head":
                    for hh in (0, 1):
                        for jc in range(JCH):
                            emit_av(au, jc, only_hh=hh)
                else:
                    for jc in range(JCH):
                        emit_av(au, jc)
                flush_pending()
            feed()
            feed()
            if su is not None:
                emit_sc(su, 2)
                emit_sc(su, 3)
                emit_sc(su, 4)
            if au is not None:
                emit_norm(au)
            continue
        if weave == "batch":
            # Coarse weave: minimize PE tile-config switches (one SC block,
            # one AV block per step).  The AV block between SC01 and SC234
            # gives ACT the lead it needs for the s2-bank reuse waits.
            if su is not None:
                emit_sc(su, 0)
                emit_sc(su, 1)
            if au is not None:
                for jc in range(JCH):
                    emit_av(au, jc)
                flush_pending()
            if su is not None:
                emit_sc(su, 2)
                emit_sc(su, 3)
                emit_sc(su, 4)
            feed()
            feed()
            if au is not None:
                emit_norm(au)
            continue
        if su is not None:
            emit_sc(su, 0)
            emit_sc(su, 1)
        if au is not None:
            emit_av(au, 0)
        feed()
        if su is not None:
            emit_sc(su, 2)
        if au is not None:
            emit_av(au, 1)
            flush_pending()
        if su is not None:
            emit_sc(su, 3)
        if au is not None:
            emit_av(au, 2)
        if su is not None:
            emit_sc(su, 4)
        feed()
        if au is not None:
            emit_av(au, 3)
            emit_av(au, 4)
            emit_norm(au)
    flush_pending()
    if opts.get("act2"):
        sink = st_pool.tile([P, 8], f32, name="edump_sink", tag="st")
        nc.vector.tensor_copy(sink, t["edump"][:, 0:8])


def _compute(nc, tc, mybir, pools, nk, t, yt_d, opts=None):
    """Production emission order: V projection, then per head-pair the Q/K
    projection chunk followed by that pair's attention, then out-proj."""
    opts = {**DEFAULT_OPTS, **(opts or {})}
    f32 = mybir.dt.float32
    xt, xkv, wqt, wkt, wvt = t["xt"], t["xkv"], t["wqt"], t["wkt"], t["wvt"]
    qt, kt, vh = t["qt"], t["kt"], t["vh"]
    pj_ps = pools["pj_ps"]
    JCH = nk // P                     # gathered-key chunks (5 for nk=640)

    # ---- V projection: v[j', o] natural layout --------------------------
    for nb2 in range(2):             # o halves (heads 8*nb2 .. 8*nb2+7)
        for mc in range(JCH):        # key chunks
            ps = pj_ps.tile([P, 512], f32, name=f"v_{nb2}_{mc}", tag="pj")
            for kc in range(KC):
                nc.tensor.matmul(
                    ps, xkv[:, kc, mc * P:(mc + 1) * P],
                    wvt[:, kc, nb2 * 512:(nb2 + 1) * 512],
                    start=(kc == 0), stop=(kc == KC - 1))
            _v_evac(nc, vh, ps, nb2, mc, opts)

    # ---- per head-pair: Q/K projection chunk then attention -------------
    def qk_group_q(g, nb2):
        def emit():
            ps = pj_ps.tile([P, 512], f32, name=f"q_{g}_{nb2}", tag="pj")
            for kc in range(KC):
                nc.tensor.matmul(
                    ps, wqt[:, kc, g * P:(g + 1) * P],
                    xt[:, kc, nb2 * 512:(nb2 + 1) * 512],
                    start=(kc == 0), stop=(kc == KC - 1))
            if opts.get("qk8"):
                _qk_evac8(nc, mybir, pools, ps, t["q8"], g, nb2 * 512, 512,
                          name=f"q8s_{g}_{nb2}")
            else:
                nc.vector.tensor_copy(
                    qt[:, g, nb2 * 512:(nb2 + 1) * 512], ps)
        return emit

    def qk_group_k(g, j0, jw):
        def emit():
            ps = pj_ps.tile([P, jw], f32, name=f"k_{g}_{j0}", tag="pj")
            for kc in range(KC):
                nc.tensor.matmul(
                    ps, wkt[:, kc, g * P:(g + 1) * P],
                    xkv[:, kc, j0:j0 + jw],
                    start=(kc == 0), stop=(kc == KC - 1))
            if opts.get("qk8"):
                _qk_evac8(nc, mybir, pools, ps, t["k8"], g, j0, jw,
                          name=f"k8s_{g}_{j0}")
            else:
                nc.vector.tensor_copy(kt[:, g, j0:j0 + jw], ps)
        return emit

    def qk_groups(g):
        yield qk_group_q(g, 0)
        yield qk_group_q(g, 1)
        yield qk_group_k(g, 0, 512)
        yield qk_group_k(g, 512, nk - 512)

    if opts.get("feed") and opts.get("emerge") and opts.get("stagepipe") \
            and nk // P == 5:
        for unit in qk_groups(0):
            unit()
        feeds = []
        for g in range(1, OC):
            feeds.extend(qk_groups(g))
        _attn_pipelined(nc, tc, mybir, pools, nk, t, opts, feeds)
    elif opts.get("feed") and opts.get("emerge"):
        pending = [None] if opts.get("norm2") else None
        for unit in qk_groups(0):
            unit()
        for g in range(OC):
            feeder = iter(qk_groups(g + 1)) if g + 1 < OC else iter(())
            _attn_pair_merged(nc, tc, mybir, pools, nk, t, g, opts, feeder,
                              pending)
            for unit in feeder:     # drain any un-fed remainder
                unit()
        if pending and pending[0] is not None:
            fin = pending[0]
            pending[0] = None
            fin()
    else:
        for g in range(OC):
            for unit in qk_groups(g):
                unit()
            _attn_pair(nc, tc, mybir, pools, nk, t, g, opts)

    _compute_proj(nc, tc, mybir, pools, nk, t, yt_d, opts)


def _qk8_perm():
    """Within-pair output-channel permutation for the fp8 DoubleRow layout:
    position p = ko*64 + h*32 + ki holds channel h*64 + ko*32 + ki."""
    chan = np.empty(P, np.int64)
    for p in range(P):
        ko, rem = divmod(p, 64)
        h, ki = divmod(rem, 32)
        chan[p] = h * 64 + ko * 32 + ki
    return np.concatenate([g * P + chan for g in range(OC)])


def _prep_inputs(x, mask, wq, wk, wv, wproj, bproj, nk=NK, qk8=False):
    """Host-side preprocessing: key gathering, transposes, scaling, casts."""
    bf = ml_dtypes.bfloat16
    scale = D ** (-0.5)
    wqt = np.ascontiguousarray((np.asarray(wq) * scale).T).astype(bf)
    wkt = np.ascontiguousarray(np.asarray(wk).T).astype(bf)
    if qk8:
        perm = _qk8_perm()
        wqt = np.ascontiguousarray(wqt[:, perm])
        wkt = np.ascontiguousarray(wkt[:, perm])
    wvt = np.ascontiguousarray(np.asarray(wv).T).astype(bf)
    wpt = np.ascontiguousarray(np.asarray(wproj).T).astype(bf)
    bb = np.ascontiguousarray(np.asarray(bproj, dtype=np.float32))
    x = np.asarray(x)
    full_mask = np.concatenate(
        [np.ones((B, 1), dtype=bool), np.asarray(mask)], axis=1)
    in_maps = []
    for b in range(B):
        xt = np.ascontiguousarray(x[b].T).astype(bf)
        idx = np.flatnonzero(full_mask[b])
        nk_b = idx.size
        assert nk_b <= nk, f"batch {b}: {nk_b} unmasked keys > padded {nk}"
        xg = np.zeros((nk, C), np.float32)
        xg[:nk_b] = x[b][idx]
        xkv = np.ascontiguousarray(xg.T).astype(bf)
        mb = np.full(nk, -1e9, np.float32)
        mb[:nk_b] = 0.0
        in_maps.append({
            "xt": xt, "xkv": xkv, "wqt": wqt, "wkt": wkt, "wvt": wvt,
            "wpt": wpt, "mb": mb, "bb": bb,
        })
    return in_maps


def get_nc(repeat=1, nk=NK, opts=None):
    key = ("nc", repeat, nk, tuple(sorted((opts or {}).items())))
    if key not in _CACHE:
        _CACHE[key] = _build_nc(repeat, nk, opts)
    return _CACHE[key]


def kernel(x, mask, wq, wk, wv, wproj, bproj):
    from concourse.bass_utils import run_bass_kernel_spmd
    full_mask = np.concatenate(
        [np.ones((B, 1), dtype=bool), np.asarray(mask)], axis=1)
    max_nk = int(full_mask.sum(axis=1).max())
    nk = NK if max_nk <= NK else ((max_nk + P - 1) // P) * P
    nc = get_nc(nk=nk)
    qk8 = bool(DEFAULT_OPTS.get("qk8")) and nk // P == 5
    in_maps = _prep_inputs(x, mask, wq, wk, wv, wproj, bproj, nk=nk, qk8=qk8)
    res = run_bass_kernel_spmd(nc, in_maps, core_ids=list(range(NCORES)))
    out = np.empty((B, N, C), np.float32)
    for b in range(B):
        out[b] = res.results[b]["yt"].T
    return out


if __name__ == "__main__":
    rng = np.random.default_rng(0)
    ins = {
        "x": rng.standard_normal((B, N, C), dtype=np.float32),
        "mask": rng.integers(0, 2, (B, N - 1)).astype(bool),
        "wq": rng.standard_normal((C, C), dtype=np.float32) * 0.02,
        "wk": rng.standard_normal((C, C), dtype=np.float32) * 0.02,
        "wv": rng.standard_normal((C, C), dtype=np.float32) * 0.02,
        "wproj": rng.standard_normal((C, C), dtype=np.float32) * 0.02,
        "bproj": rng.standard_normal((C,), dtype=np.float32) * 0.02,
    }
    o = kernel(**ins)
    print(o.shape, o.dtype)

